# revision 26
# baseline (speedup 1.0000x reference)
"""Trainium2 Bass kernel for nn_ConcatAttentionHeads (B=4, S=2048, NHID=1024,
NHEAD=16, HDIM=64).

Sharding: 8 cores; core c owns (batch b=c//2, head-half hh=c%2, i.e. heads
hh*8..hh*8+8). Each core computes q/k/v for all 2048 tokens of its batch for
its 8 heads, full attention for those heads, and a PARTIAL output projection
(contraction over its 512 concat channels). Host sums the two partials per
batch and adds WO_b.

Engine budget per core: PE ~302us of matmul slots (scores 256 row-tiled
concurrent pairs, AV 512, projections 512), ACT ~285us of exp stream (256
tiles of [128,1024]); DVE/GPSIMD well under. The schedule aims to keep the
ACT exp stream dense from ~25us on:
  - DMAs ordered by first use (wv, wk0/wq0, x chunks, then the rest).
  - Prologue: kT[0] (pipelined behind the x DMAs), all of v8, qT[0] first
    half; first score pair issues ~25us in.
  - Remaining projections + output projection run as fillers popped at a
    fixed rate (2 matmuls/step) so the PE never starves ACT and never
    front-loads the fillers.
  - Normalize: reciprocal_approx_fast on DVE + partition_broadcast on the
    (otherwise idle) GPSIMD + one fused PSUM-read multiply on DVE. No DMA
    broadcasts, no 3.3us full reciprocals.
"""
from collections import deque

import numpy as np

import concourse.bass as bass
import concourse.mybir as mybir
import concourse.tile as tile
from concourse import bacc
from concourse import bass_utils
from concourse import library_config

F32 = mybir.dt.float32
BF16 = mybir.dt.bfloat16
AF = mybir.ActivationFunctionType

P = 128
S = 2048          # tokens (queries == keys)
NHID = 1024
NHC = 8           # heads per core
NPAIR = 4         # head pairs per core
D = 64
NKC = NHID // P   # 8 contraction chunks
NSTC = S // P     # 16 key chunks
NSQ = S // 512    # 4 query chunks of 512
CCH = NHC * D     # 512 concat channels per core
SCALE = 1.0 / np.sqrt(D)

_PROGRAM_CACHE = {}


def build_program():
    if "nc" in _PROGRAM_CACHE:
        return _PROGRAM_CACHE["nc"]

    nc = bacc.Bacc("TRN2", target_bir_lowering=False, debug=False)

    xT_d = nc.dram_tensor("xT", [NHID, S], BF16, kind="ExternalInput")
    wq_d = nc.dram_tensor("wq", [NPAIR, P, NKC * P], BF16,
                          kind="ExternalInput")
    wk_d = nc.dram_tensor("wk", [NPAIR, P, NKC * P], BF16,
                          kind="ExternalInput")
    wv_d = nc.dram_tensor("wv", [P, NKC * CCH], BF16, kind="ExternalInput")
    wo_d = nc.dram_tensor("wo", [CCH, NHID], BF16, kind="ExternalInput")
    bq_d = nc.dram_tensor("bq", [P, NPAIR], F32, kind="ExternalInput")
    bk_d = nc.dram_tensor("bk", [P, NPAIR], F32, kind="ExternalInput")
    bvb_d = nc.dram_tensor("bvb", [1, CCH], F32, kind="ExternalInput")
    out_d = nc.dram_tensor("out", [NHID, S], F32, kind="ExternalOutput")

    with tile.TileContext(nc) as tc:
        with (
            tc.tile_pool(name="main", bufs=1) as mp,
            tc.tile_pool(name="ps", bufs=1, space="PSUM") as ps,
        ):
            # partition_broadcast lives in the gpsimd `attn` ucode library
            # (standard/index 0 is the boot default; CoreSim doesn't model
            # library state but hardware does).
            nc.gpsimd.load_library(library_config.attn)

            # ---- constants ----
            bq_sb = mp.tile([P, NPAIR], F32, tag="bq")
            bk_sb = mp.tile([P, NPAIR], F32, tag="bk")
            bv1_sb = mp.tile([1, CCH], F32, tag="bv1")
            bvb_sb = mp.tile([P, CCH], F32, tag="bvb")
            nc.sync.dma_start(bq_sb, bq_d.ap())
            nc.sync.dma_start(bk_sb, bk_d.ap())
            nc.sync.dma_start(bv1_sb, bvb_d.ap())
            nc.gpsimd.partition_broadcast(bvb_sb, bv1_sb)

            # ---- DMAs in first-use order ----
            wq_sb = [mp.tile([P, NKC, P], BF16, tag="wq", bufs=NPAIR,
                             name=f"wq{p}") for p in range(NPAIR)]
            wk_sb = [mp.tile([P, NKC, P], BF16, tag="wk", bufs=NPAIR,
                             name=f"wk{p}") for p in range(NPAIR)]
            nc.sync.dma_start(
                wk_sb[0], wk_d.ap()[0].rearrange("p (c m) -> p c m", c=NKC))
            nc.sync.dma_start(
                wq_sb[0], wq_d.ap()[0].rearrange("p (c m) -> p c m", c=NKC))

            wv_sb = mp.tile([P, NKC, CCH], BF16, tag="wv")
            nc.sync.dma_start(
                wv_sb, wv_d.ap().rearrange("p (c m) -> p c m", c=NKC))
            xtA = [mp.tile([P, 1024], BF16, tag="xtA", bufs=NKC,
                           name=f"xtA{kc}") for kc in range(NKC)]
            xtB = [mp.tile([P, 1024], BF16, tag="xtB", bufs=NKC,
                           name=f"xtB{kc}") for kc in range(NKC)]
            # x rides the (idle-at-start) Vector queue so its issue slots
            # run in parallel with the weight DMAs on the Sync queue
            for kc in range(NKC):
                nc.scalar.dma_start(xtA[kc],
                                    xT_d.ap()[kc * P:(kc + 1) * P, 0:1024])
            for kc in range(NKC):
                nc.scalar.dma_start(xtB[kc],
                                    xT_d.ap()[kc * P:(kc + 1) * P, 1024:2048])

            def xh(kc, base):
                """xt slice [kc][:, base:base+512] across the half tiles."""
                if base < 1024:
                    return xtA[kc][:, base:base + 512]
                return xtB[kc][:, base - 1024:base - 1024 + 512]

            def xp(kc, stc):
                """xt slice [kc][:, stc*128:(stc+1)*128]."""
                if stc < 8:
                    return xtA[kc][:, stc * P:(stc + 1) * P]
                return xtB[kc][:, (stc - 8) * P:(stc - 7) * P]
            for p in range(1, NPAIR):
                nc.sync.dma_start(
                    wk_sb[p], wk_d.ap()[p].rearrange("p (c m) -> p c m",
                                                     c=NKC))
                nc.sync.dma_start(
                    wq_sb[p], wq_d.ap()[p].rearrange("p (c m) -> p c m",
                                                     c=NKC))
            wo_sb = []
            for ci in range(NPAIR):
                w = mp.tile([P, NHID], BF16, tag="wo", bufs=NPAIR,
                            name=f"wo{ci}")
                nc.sync.dma_start(w, wo_d.ap()[ci * P:(ci + 1) * P, :])
                wo_sb.append(w)

            # ---- persistent SBUF products ----
            qT = [mp.tile([P, S], BF16, tag="qT", bufs=NPAIR, name=f"qT{p}")
                  for p in range(NPAIR)]
            kT = [mp.tile([P, S], BF16, tag="kT", bufs=NPAIR, name=f"kT{p}")
                  for p in range(NPAIR)]
            v8 = [mp.tile([P, NHC * 65], BF16, tag="v8", bufs=NSTC,
                          name=f"v8_{stc}") for stc in range(NSTC)]
            cc4 = [mp.tile([P, S], BF16, tag="cc", bufs=NPAIR,
                           name=f"cc{p}") for p in range(NPAIR)]

            # ---- emission helpers (yield granularity: ~1 matmul) ----
            def emit_qk_half(w_sb, b_sb, dst, p, h4):
                """One q or k projection for one 512-token half; 8 MMs."""
                pj = ps.tile([P, 512], F32, tag="pj", bufs=2,
                             name=f"qk{p}h{h4}")
                base = h4 * 512
                for kc in range(NKC):
                    nc.tensor.matmul(
                        pj, w_sb[p][:, kc, :], xh(kc, base),
                        start=(kc == 0), stop=(kc == NKC - 1))
                    if kc == NKC - 1:
                        nc.vector.tensor_scalar_add(
                            dst[p][:, base:base + 512], pj, b_sb[:, p:p + 1])
                    yield

            def emit_qk_units(w_sb, b_sb, dst, p, halves):
                for h4 in halves:
                    yield from emit_qk_half(w_sb, b_sb, dst, p, h4)

            def emit_v_unit(stc):
                """v8[stc] = x[:, stc]ᵀ @ Wv + bv, plus ones column; 8 MMs."""
                pv = ps.tile([P, 512], F32, tag="pj", bufs=2, name=f"pv{stc}")
                tj = v8[stc].rearrange("p (j e) -> p j e", e=65)
                for kc in range(NKC):
                    nc.tensor.matmul(
                        pv, xp(kc, stc), wv_sb[:, kc, :],
                        start=(kc == 0), stop=(kc == NKC - 1))
                    if kc == NKC - 1:
                        nc.vector.memset(tj[:, :, 64:65], 1.0)
                        nc.vector.tensor_add(
                            tj[:, :, 0:64],
                            pv.rearrange("p (j e) -> p j e", e=64),
                            bvb_sb.rearrange("p (j e) -> p j e", e=64))
                    yield

            # out-projection in two passes so its 128 MMs spread across the
            # whole run instead of clustering behind the last pair: pairs
            # 0-2 accumulate into a bf16 SBUF partial as soon as p2's
            # normalize lands; the p3 pass adds the last matmul and stores.
            acc = [mp.tile([P, NHID // P, 512], BF16, tag="acc", bufs=NSQ,
                           name=f"acc{s}") for s in range(NSQ)]

            def emit_outproj3(s):
                """Pairs 0-2 partial out-projection for block s; 24 MMs."""
                for oc in range(NHID // P):
                    po = ps.tile([P, 512], F32, tag="pj", bufs=2,
                                 name=f"p3_{s}{oc}")
                    for ci in range(3):
                        nc.tensor.matmul(
                            po, wo_sb[ci][:, oc * P:(oc + 1) * P],
                            cc4[ci][:, s * 512:(s + 1) * 512],
                            start=(ci == 0), stop=(ci == 2))
                        if ci == 2:
                            nc.vector.tensor_copy(acc[s][:, oc, :], po)
                        yield

            def emit_outproj_last(s):
                """Pair-3 matmul + partial add + store for block s; 8 MMs."""
                for oc in range(NHID // P):
                    po = ps.tile([P, 512], F32, tag="pj", bufs=2,
                                 name=f"pl_{s}{oc}")
                    nc.tensor.matmul(
                        po, wo_sb[3][:, oc * P:(oc + 1) * P],
                        cc4[3][:, s * 512:(s + 1) * 512],
                        start=True, stop=True)
                    ot = mp.tile([P, 512], F32, tag="oout", bufs=4)
                    nc.vector.tensor_add(ot, po, acc[s][:, oc, :])
                    nc.sync.dma_start(
                        out_d.ap()[oc * P:(oc + 1) * P,
                                   s * 512:(s + 1) * 512], ot)
                    yield

            fillers = deque()
            TOTAL_STEPS = NPAIR * NSQ * NSTC
            pace = {"pending": 0.0, "step": 0, "carry": 0.0, "popped": 0}

            def add_filler(gen, n_mms):
                fillers.append(gen)
                pace["pending"] += n_mms

            def pop_one():
                while fillers:
                    g = fillers[0]
                    try:
                        next(g)
                        pace["pending"] -= 1
                        pace["popped"] += 1
                        return True
                    except StopIteration:
                        fillers.popleft()
                return False

            def pop_filler_paced():
                pace["step"] += 1
                left = TOTAL_STEPS - pace["step"]
                if left <= 0:
                    return
                # floor keeps the q/k generators ahead of their consuming
                # pair's first group (correctness: emission order defines
                # the dependency order)
                rate = max(pace["pending"] / left, 1.6)
                pace["carry"] += rate
                n = int(pace["carry"])
                while n > 0:
                    if not pop_one():
                        break
                    pace["carry"] -= 1
                    n -= 1
                pace["carry"] = min(pace["carry"], 4.0)

            def force_drain(min_popped):
                while pace["popped"] < min_popped:
                    if not pop_one():
                        break

            # ---- prologue, pipelined behind the two x DMA halves:
            # kT[0] halves 0-1 + v8[0..7] (token half 0), then halves 2-3 +
            # v8[8..15], then qT[0] half 0 ----
            for _ in emit_qk_units(wk_sb, bk_sb, kT, 0, (0, 1)):
                pass
            for stc in range(NSTC // 2):
                for _ in emit_v_unit(stc):
                    pass
            for _ in emit_qk_units(wk_sb, bk_sb, kT, 0, (2, 3)):
                pass
            for stc in range(NSTC // 2, NSTC):
                for _ in emit_v_unit(stc):
                    pass
            for _ in emit_qk_half(wq_sb, bq_sb, qT, 0, 0):
                pass

            # remaining projections become fillers (consumption order:
            # qT0 halves 1-3, then q/k for pairs 1-3 in group order)
            add_filler(emit_qk_units(wq_sb, bq_sb, qT, 0, (1, 2, 3)), 24)
            for p in range(1, NPAIR):
                add_filler(emit_qk_units(wk_sb, bk_sb, kT, p,
                                         (0, 1, 2, 3)), 32)
                add_filler(emit_qk_units(wq_sb, bq_sb, qT, p,
                                         (0, 1, 2, 3)), 32)

            # ---- attention groups, software-pipelined across boundaries ----
            # The last two AV pairs + the normalize of group g run during the
            # first steps of group g+1, so the next group's score/exp stream
            # never sits behind AV(15)'s wait on exp(15). The op PSUM pair is
            # freed by two cheap DVE copies (den row + o block → SBUF); the
            # reciprocal/broadcast/multiply read those copies lazily.
            def make_tail(p, s, op, e_tiles, av, after_norm=None):
                osbs = []

                def tail_av_a():
                    av(NSTC - 2)

                dens = []

                def tail_av_stop_copies():
                    av(NSTC - 1)
                    # den must land at partition 0: the custom-DVE
                    # reciprocal misreads any partition-offset input on HW
                    for h in range(2):
                        den = mp.tile([1, 512], F32, tag="den", bufs=2)
                        nc.vector.tensor_copy(den, op[h][64:65, :])
                        dens.append(den)
                        osb = mp.tile([D, 512], F32, tag="osb", bufs=2)
                        nc.vector.tensor_copy(osb, op[h][0:D, :])
                        osbs.append(osb)

                def tail_norm():
                    for h in range(2):
                        rec = mp.tile([1, 512], F32, tag="rec", bufs=2)
                        nc.vector.reciprocal_approx_fast(rec, dens[h])
                        rbc = mp.tile([D, 512], F32, tag="rbc", bufs=2)
                        nc.gpsimd.partition_broadcast(rbc, rec)
                        nc.vector.tensor_mul(
                            cc4[p][h * D:(h + 1) * D,
                                   s * 512:(s + 1) * 512],
                            osbs[h], rbc)
                    # only now is cc4[p][.., s] written in program order —
                    # safe to let dependents (outproj) into the filler deque
                    if after_norm is not None:
                        after_norm()

                return deque([tail_av_a, tail_av_stop_copies, tail_norm])

            pending_tail = deque()

            def attention_group(p, s):
                nonlocal pending_tail
                op = [ps.tile([65, 512], F32, tag="av", bufs=2,
                              name=f"op{p}_{s}_{h}") for h in range(2)]
                e_tiles = [None] * NSTC

                def av(stc):
                    for h in range(2):
                        lv = v8[stc][:, (2 * p + h) * 65:(2 * p + h) * 65 + 65]
                        nc.tensor.matmul(
                            op[h][:, :], lv,
                            e_tiles[stc][:, h * 512:(h + 1) * 512],
                            start=(stc == 0), stop=(stc == NSTC - 1))

                for stc in range(NSTC):
                    e = mp.tile([P, 1024], BF16, tag="expT", bufs=6)
                    if stc == 0:
                        # first step borrows the pj banks so these scores
                        # don't WAR-wait on the previous group's exp(14)
                        # (sc ring depth is only 2); exp splits in two
                        for h in range(2):
                            sch = ps.tile([P, 512], F32, tag="pj", bufs=2,
                                          name=f"sc0_{p}_{s}_{h}")
                            nc.tensor.matmul(
                                sch,
                                kT[p][h * D:(h + 1) * D,
                                      stc * P:(stc + 1) * P],
                                qT[p][h * D:(h + 1) * D,
                                      s * 512:(s + 1) * 512],
                                start=True, stop=True)
                            nc.scalar.activation(
                                e[:, h * 512:(h + 1) * 512], sch,
                                AF.Exp, scale=SCALE)
                    else:
                        sc = ps.tile([P, 1024], F32, tag="sc", bufs=2,
                                     name=f"sc{p}_{s}_{stc}")
                        for h in range(2):
                            nc.tensor.matmul(
                                sc[:, h * 512:(h + 1) * 512],
                                kT[p][h * D:(h + 1) * D,
                                      stc * P:(stc + 1) * P],
                                qT[p][h * D:(h + 1) * D,
                                      s * 512:(s + 1) * 512],
                                start=True, stop=True)
                        nc.scalar.activation(e, sc, AF.Exp, scale=SCALE)
                    e_tiles[stc] = e
                    if pending_tail:
                        pending_tail.popleft()()
                    if stc >= 2:
                        av(stc - 2)
                    pop_filler_paced()
                after_norm = None
                if p == 2:
                    after_norm = (lambda s=s:
                                  add_filler(emit_outproj3(s), 24))
                elif p == NPAIR - 1:
                    after_norm = (lambda s=s:
                                  add_filler(emit_outproj_last(s), 8))
                pending_tail = make_tail(p, s, op, e_tiles, av, after_norm)

            for p in range(NPAIR):
                if p >= 1:
                    # pair p's q/k emission must precede its first group
                    force_drain(24 + 64 * p)
                for s in range(NSQ):
                    attention_group(p, s)

            # ---- drain the last group's tail and remaining fillers ----
            while pending_tail:
                pending_tail.popleft()()
            while fillers:
                g = fillers.popleft()
                for _ in g:
                    pass

    nc.compile()
    _PROGRAM_CACHE["nc"] = nc
    return nc


def _prep_inputs(x, Wq, bq, Wk, bk, Wv, bv, WO_w, WO_b):
    """Host-side sharding/layout prep -> list of 8 per-core input maps."""
    import ml_dtypes
    BF = ml_dtypes.bfloat16
    x = np.asarray(x, dtype=np.float32)
    Wq = np.asarray(Wq, dtype=np.float32)
    Wk = np.asarray(Wk, dtype=np.float32)
    Wv = np.asarray(Wv, dtype=np.float32)
    bq = np.asarray(bq, dtype=np.float32)
    bk = np.asarray(bk, dtype=np.float32)
    bv = np.asarray(bv, dtype=np.float32)
    WO_w = np.asarray(WO_w, dtype=np.float32)

    xts = [np.ascontiguousarray(x[b].T).astype(BF) for b in range(4)]
    woT = np.ascontiguousarray(WO_w.T).astype(BF)  # [in, out]

    half = {}
    for hh in range(2):
        h0 = hh * NHC
        def to_pcm(w):
            # [NHID, M] -> [P, NKC*M] with partition = nhid within chunk
            m = w.shape[1]
            return np.ascontiguousarray(
                w.reshape(NKC, P, m).transpose(1, 0, 2).reshape(P, NKC * m))

        wq_p = np.stack(
            [to_pcm(np.concatenate(
                [Wq[h0 + 2 * p], Wq[h0 + 2 * p + 1]], axis=1))
             for p in range(NPAIR)]).astype(BF)
        wk_p = np.stack(
            [to_pcm(np.concatenate(
                [Wk[h0 + 2 * p], Wk[h0 + 2 * p + 1]], axis=1))
             for p in range(NPAIR)]).astype(BF)
        wv_g = to_pcm(np.concatenate(
            [Wv[h0 + j] for j in range(NHC)], axis=1)).astype(BF)
        bq_p = np.stack(
            [np.concatenate([bq[h0 + 2 * p], bq[h0 + 2 * p + 1]])
             for p in range(NPAIR)], axis=1).astype(np.float32)
        bk_p = np.stack(
            [np.concatenate([bk[h0 + 2 * p], bk[h0 + 2 * p + 1]])
             for p in range(NPAIR)], axis=1).astype(np.float32)
        bvb = bv[h0:h0 + NHC].reshape(1, -1).astype(np.float32)
        half[hh] = {
            "wq": wq_p, "wk": wk_p, "wv": wv_g,
            "wo": np.ascontiguousarray(woT[hh * CCH:(hh + 1) * CCH, :]),
            "bq": bq_p, "bk": bk_p, "bvb": bvb,
        }

    in_maps = []
    for c in range(8):
        b, hh = c // 2, c % 2
        m = dict(half[hh])
        m["xT"] = xts[b]
        in_maps.append(m)
    return in_maps


def kernel(x, Wq, bq, Wk, bk, Wv, bv, WO_w, WO_b, _trace=False, _tmpdir=None):
    nc = build_program()
    in_maps = _prep_inputs(x, Wq, bq, Wk, bk, Wv, bv, WO_w, WO_b)
    res = bass_utils.run_bass_kernel_spmd(
        nc, in_maps, core_ids=list(range(8)), trace=_trace, tmpdir=_tmpdir
    )
    B = 4
    WO_b = np.asarray(WO_b, dtype=np.float32)
    out = np.empty((B, NHID, S), dtype=np.float32)
    for b in range(B):
        out[b] = (res.results[2 * b]["out"] + res.results[2 * b + 1]["out"]
                  + WO_b[:, None])
    kernel.last_results = res
    return out
